# revision 15
# baseline (speedup 1.0000x reference)
"""GF(2) linear block encoder c = (b @ G) mod 2 on 8 TRN2 NeuronCores.

Strategy:
  - Data-parallel: shard b rows (32768 -> 8 x 4096), replicate G.
  - Bits {0,1} are exact in fp8-e4m3 and products accumulate exactly in
    fp32 PSUM (sums <= 1024 << 2^24), so the GF(2) matmul is computed as
    an fp8 DoubleRow matmul (K=256 per MM) at ~2x bf16 throughput.
  - mod 2 is a single DVE tensor_scalar(mod, 2.0) pass PSUM->SBUF uint8.
  - Host packs b into [128, 8, M] (k = s*128 + p) transposed layout and
    casts to fp8; output uint8 is cast back to int32 on host.
"""

import os
import sys

import numpy as np

if "/opt/trn_rl_repo" not in sys.path:
    sys.path.insert(0, "/opt/trn_rl_repo")

import ml_dtypes

B_ROWS = 32768
K_MSG = 1024
N_CODE = 2048
NCORES = 8
M = B_ROWS // NCORES  # 4096 rows per core
KS = K_MSG // 128     # 8 k-subtiles of 128
KP = KS // 2          # 4 DoubleRow k-pair steps (K=256 each)
MT = M // 128         # 32 m-tiles
NT = N_CODE // 512    # 4 n-tiles
MC = 8                # b DMA chunks along m
MCW = M // MC         # 512 m per chunk

F8 = ml_dtypes.float8_e4m3

_NC_CACHE = None


def _build_bass():
    import concourse.bacc as bacc
    import concourse.mybir as mybir
    from concourse import tile

    nc = bacc.Bacc("TRN2", target_bir_lowering=False, debug=False)

    bt = nc.dram_tensor("bt", [128, KS, M], mybir.dt.float8e4, kind="ExternalInput")
    g = nc.dram_tensor("g", [128, KS, N_CODE], mybir.dt.float8e4, kind="ExternalInput")
    c = nc.dram_tensor("c", [M, N_CODE], mybir.dt.int32, kind="ExternalOutput")

    dr = mybir.MatmulPerfMode.DoubleRow

    with tile.TileContext(nc) as tc:
        with (
            tc.tile_pool(name="persist", bufs=1) as persist,
            tc.tile_pool(name="psum", bufs=8, space="PSUM") as psum_pool,
            tc.tile_pool(name="couts", bufs=4) as couts,
            tc.tile_pool(name="i32s", bufs=8) as i32s,
        ):
            # G resident: 4 chunks of [128, 2, N] (k-pair each), contiguous DMA
            g_tiles = []
            for kp in range(KP):
                gt = persist.tile([128, 2, N_CODE], mybir.dt.float8e4, tag=f"g{kp}")
                nc.sync.dma_start(out=gt, in_=g[:, 2 * kp : 2 * kp + 2, :])
                g_tiles.append(gt)

            # b resident: 8 chunks of [128, KS, 512] along m so compute can
            # start after the first chunk lands
            b_tiles = []
            for mc in range(MC):
                btile = persist.tile([128, KS, MCW], mybir.dt.float8e4, tag=f"b{mc}")
                nc.sync.dma_start(
                    out=btile, in_=bt[:, :, mc * MCW : (mc + 1) * MCW]
                )
                b_tiles.append(btile)

            # output viewed as [MC groups, 128 p, 4 j, N]: row m = mc*512+j*128+p
            c_view = c.rearrange("(mc j p) n -> mc p j n", j=MT // MC, p=128)
            JT = MT // MC  # 4 m-tiles per output group

            for mt in range(MT):
                mc = mt // JT
                j = mt % JT
                m0 = j * 128
                if j == 0:
                    c_sb = couts.tile([128, JT, N_CODE], mybir.dt.int32)
                for nt in range(NT):
                    ps = psum_pool.tile([128, 512], mybir.dt.float32)
                    for kp in range(KP):
                        nc.tensor.matmul(
                            ps,
                            b_tiles[mc][:, 2 * kp : 2 * kp + 2, m0 : m0 + 128],
                            g_tiles[kp][:, :, nt * 512 : (nt + 1) * 512],
                            start=(kp == 0),
                            stop=(kp == KP - 1),
                            perf_mode=dr,
                        )
                    # mod 2 = LSB: ACT casts psum fp32 -> int32, DVE ands with 1
                    t32 = i32s.tile([128, 512], mybir.dt.int32)
                    nc.scalar.activation(t32, ps, mybir.ActivationFunctionType.Copy)
                    nc.vector.tensor_scalar(
                        out=c_sb[:, j, nt * 512 : (nt + 1) * 512],
                        in0=t32,
                        scalar1=1,
                        scalar2=None,
                        op0=mybir.AluOpType.bitwise_and,
                    )
                if j == JT - 1:
                    # one out-DMA per group, on its own SWDGE lane, so no
                    # DMA-lane ring wait is ever needed (a DMA descriptor can
                    # carry only a single sync wait - the DVE data wait)
                    nc.gpsimd.dma_start(out=c_view[mc], in_=c_sb)

    nc.finalize()  # bacc: regalloc + event-semaphore legalization
    return nc


def _get_nc():
    global _NC_CACHE
    if _NC_CACHE is None:
        _NC_CACHE = _build_bass()
    return _NC_CACHE


def _pack_inputs(b, G):
    b8 = np.asarray(b).astype(np.uint8)
    G8 = np.asarray(G).astype(np.uint8)
    # [p, s, n] with k = s*128 + p
    g_f8 = G8.reshape(KS, 128, N_CODE).transpose(1, 0, 2).astype(F8, order="C")
    bts = []
    for core in range(NCORES):
        sh = b8[core * M : (core + 1) * M]          # [M, K]
        bt = sh.T.reshape(KS, 128, M).transpose(1, 0, 2)  # [p, s, m]
        bts.append(bt.astype(F8, order="C"))
    return bts, g_f8


def kernel(b, G, trace=False, **run_kwargs):
    from concourse.bass_utils import run_bass_kernel_spmd

    nc = _get_nc()
    bts, g_f8 = _pack_inputs(b, G)
    in_maps = [{"bt": bts[i], "g": g_f8} for i in range(NCORES)]
    res = run_bass_kernel_spmd(
        nc, in_maps, core_ids=list(range(NCORES)), trace=trace, **run_kwargs
    )
    out = np.concatenate([res.results[i]["c"] for i in range(NCORES)], axis=0)
    if out.dtype != np.int32:
        out = out.astype(np.int32)
    if trace:
        kernel.last_results = res
    return out


kernel.last_results = None


# revision 17
# speedup vs baseline: 1.1488x; 1.1488x over previous
"""GF(2) linear block encoder c = (b @ G) mod 2 on 8 TRN2 NeuronCores.

Strategy:
  - Data-parallel: shard b rows (32768 -> 8 x 4096), replicate G.
  - Bits {0,1} are exact in fp8-e4m3 and products accumulate exactly in
    fp32 PSUM (sums <= 1024 << 2^24), so the GF(2) matmul is computed as
    an fp8 DoubleRow matmul (K=256 per MM) at ~2x bf16 throughput.
  - mod 2 is a single DVE tensor_scalar(mod, 2.0) pass PSUM->SBUF uint8.
  - Host packs b into [128, 8, M] (k = s*128 + p) transposed layout and
    casts to fp8; output uint8 is cast back to int32 on host.
"""

import os
import sys

import numpy as np

if "/opt/trn_rl_repo" not in sys.path:
    sys.path.insert(0, "/opt/trn_rl_repo")

import ml_dtypes

B_ROWS = 32768
K_MSG = 1024
N_CODE = 2048
NCORES = 8
M = B_ROWS // NCORES  # 4096 rows per core
KS = K_MSG // 128     # 8 k-subtiles of 128
KP = KS // 2          # 4 DoubleRow k-pair steps (K=256 each)
MT = M // 128         # 32 m-tiles
NT = N_CODE // 512    # 4 n-tiles
MC = 8                # b DMA chunks along m
MCW = M // MC         # 512 m per chunk

F8 = ml_dtypes.float8_e4m3

_NC_CACHE = None


def _build_bass():
    import concourse.bacc as bacc
    import concourse.mybir as mybir
    from concourse import tile

    nc = bacc.Bacc("TRN2", target_bir_lowering=False, debug=False)

    bt = nc.dram_tensor("bt", [128, KS, M], mybir.dt.float8e4, kind="ExternalInput")
    g = nc.dram_tensor("g", [128, KS, N_CODE], mybir.dt.float8e4, kind="ExternalInput")
    c = nc.dram_tensor("c", [M, N_CODE], mybir.dt.int32, kind="ExternalOutput")

    dr = mybir.MatmulPerfMode.DoubleRow

    with tile.TileContext(nc) as tc:
        with (
            tc.tile_pool(name="persist", bufs=1) as persist,
            tc.tile_pool(name="psum", bufs=2, space="PSUM") as psum_pool,
            tc.tile_pool(name="couts", bufs=4) as couts,
        ):
            # G resident: 4 chunks of [128, 2, N] (k-pair each), contiguous DMA
            g_tiles = []
            for kp in range(KP):
                gt = persist.tile([128, 2, N_CODE], mybir.dt.float8e4, tag=f"g{kp}")
                nc.sync.dma_start(out=gt, in_=g[:, 2 * kp : 2 * kp + 2, :])
                g_tiles.append(gt)

            # b resident: 8 chunks of [128, KS, 512] along m so compute can
            # start after the first chunk lands
            b_tiles = []
            for mc in range(MC):
                btile = persist.tile([128, KS, MCW], mybir.dt.float8e4, tag=f"b{mc}")
                nc.sync.dma_start(
                    out=btile, in_=bt[:, :, mc * MCW : (mc + 1) * MCW]
                )
                b_tiles.append(btile)

            # output viewed as [MC groups, 128 p, 4 j, N]: row m = mc*512+j*128+p
            c_view = c.rearrange("(mc j p) n -> mc p j n", j=MT // MC, p=128)
            JT = MT // MC  # 4 m-tiles per output group

            for mt in range(MT):
                mc = mt // JT
                j = mt % JT
                m0 = j * 128
                if j == 0:
                    c_sb = couts.tile([128, JT, N_CODE], mybir.dt.int32)
                ps = psum_pool.tile([128, N_CODE], mybir.dt.float32)  # 4 banks
                for nt in range(NT):
                    for kp in range(KP):
                        nc.tensor.matmul(
                            ps[:, nt * 512 : (nt + 1) * 512],
                            b_tiles[mc][:, 2 * kp : 2 * kp + 2, m0 : m0 + 128],
                            g_tiles[kp][:, :, nt * 512 : (nt + 1) * 512],
                            start=(kp == 0),
                            stop=(kp == KP - 1),
                            perf_mode=dr,
                        )
                # mod 2 = LSB: one big ACT cast psum fp32 -> int32, then one
                # in-place DVE and-with-1 (big tiles amortize per-inst cost)
                nc.scalar.activation(
                    c_sb[:, j, :], ps, mybir.ActivationFunctionType.Copy
                )
                nc.vector.tensor_scalar(
                    out=c_sb[:, j, :],
                    in0=c_sb[:, j, :],
                    scalar1=1,
                    scalar2=None,
                    op0=mybir.AluOpType.bitwise_and,
                )
                if j == JT - 1:
                    # one out-DMA per group, on its own SWDGE lane, so no
                    # DMA-lane ring wait is ever needed (a DMA descriptor can
                    # carry only a single sync wait - the DVE data wait)
                    nc.gpsimd.dma_start(out=c_view[mc], in_=c_sb)

    nc.finalize()  # bacc: regalloc + event-semaphore legalization
    return nc


def _get_nc():
    global _NC_CACHE
    if _NC_CACHE is None:
        _NC_CACHE = _build_bass()
    return _NC_CACHE


def _pack_inputs(b, G):
    b8 = np.asarray(b).astype(np.uint8)
    G8 = np.asarray(G).astype(np.uint8)
    # [p, s, n] with k = s*128 + p
    g_f8 = G8.reshape(KS, 128, N_CODE).transpose(1, 0, 2).astype(F8, order="C")
    bts = []
    for core in range(NCORES):
        sh = b8[core * M : (core + 1) * M]          # [M, K]
        bt = sh.T.reshape(KS, 128, M).transpose(1, 0, 2)  # [p, s, m]
        bts.append(bt.astype(F8, order="C"))
    return bts, g_f8


def kernel(b, G, trace=False, **run_kwargs):
    from concourse.bass_utils import run_bass_kernel_spmd

    nc = _get_nc()
    bts, g_f8 = _pack_inputs(b, G)
    in_maps = [{"bt": bts[i], "g": g_f8} for i in range(NCORES)]
    res = run_bass_kernel_spmd(
        nc, in_maps, core_ids=list(range(NCORES)), trace=trace, **run_kwargs
    )
    out = np.concatenate([res.results[i]["c"] for i in range(NCORES)], axis=0)
    if out.dtype != np.int32:
        out = out.astype(np.int32)
    if trace:
        kernel.last_results = res
    return out


kernel.last_results = None
